# revision 2
# baseline (speedup 1.0000x reference)
import numpy as np

import concourse.bacc as bacc
import concourse.tile as tile
import concourse.mybir as mybir
from concourse.bass_utils import run_bass_kernel_spmd

F32 = mybir.dt.float32
F32R = mybir.dt.float32r

B = 4
N = 2048
PD = 512
CD = 128
ID = 512
OUT_D = 512
H_PER_CORE = 4
DH = 64
SCALE = 0.125          # dim_head ** -0.5
NT = 4                 # n chunks of 512
NP = 4                 # pd chunks of 128
NKT = 16               # key tiles of 128
VW = 65 * H_PER_CORE   # vtile columns per key tile: 4 x (64 V | 1 one)

_NC = None
LAST_EXEC_NS = None


def _build():
    nc = bacc.Bacc("TRN2", target_bir_lowering=False, debug=False, num_devices=8)
    XT = nc.declare_dram_parameter("XT", [PD, N], F32, isOutput=False)
    CT = nc.declare_dram_parameter("CT", [CD, N], F32, isOutput=False)
    Wqp = nc.declare_dram_parameter("Wqp", [128, 1024], F32, isOutput=False)
    Wkp = nc.declare_dram_parameter("Wkp", [128, 1024], F32, isOutput=False)
    Wv = nc.declare_dram_parameter("Wv", [128, 1024], F32, isOutput=False)
    Wqc = nc.declare_dram_parameter("Wqc", [CD, 256], F32, isOutput=False)
    Wkc = nc.declare_dram_parameter("Wkc", [CD, 256], F32, isOutput=False)
    Wo = nc.declare_dram_parameter("Wo", [128, 1024], F32, isOutput=False)
    Y = nc.declare_dram_parameter("Y", [N, OUT_D], F32, isOutput=True)

    MIN = mybir.AluOpType.min
    MAX = mybir.AluOpType.max
    MULT = mybir.AluOpType.mult
    ADD = mybir.AluOpType.add
    EXP = mybir.ActivationFunctionType.Exp

    with tile.TileContext(nc) as tc, \
         tc.tile_pool(name="persist", bufs=1) as pp, \
         tc.tile_pool(name="work", bufs=2) as wk:
        xt = [pp.tile([128, N], F32R, name=f"xt{p}", tag=f"xt{p}") for p in range(NP)]
        ct = pp.tile([128, N], F32R, name="ct", tag="ct")
        wqp = pp.tile([128, 1024], F32R, name="wqp", tag="wqp")
        wkp = pp.tile([128, 1024], F32R, name="wkp", tag="wkp")
        wv = pp.tile([128, 1024], F32R, name="wv", tag="wv")
        wqc = pp.tile([128, 256], F32R, name="wqc", tag="wqc")
        wkc = pp.tile([128, 256], F32R, name="wkc", tag="wkc")
        wo = pp.tile([128, 1024], F32R, name="wo", tag="wo")
        qcat = [pp.tile([128, N], F32R, name=f"qcat{h}", tag=f"qcat{h}")
                for h in range(H_PER_CORE)]
        kcat = [pp.tile([128, N], F32R, name=f"kcat{h}", tag=f"kcat{h}")
                for h in range(H_PER_CORE)]
        vtile = pp.tile([128, NKT * VW], F32R, name="vtile", tag="vtile")
        ocat = [pp.tile([128, N], F32R, name=f"ocat{j}", tag=f"ocat{j}")
                for j in range(2)]

        nc.sync.dma_start(wkp[:], Wkp[:].bitcast(F32R))
        nc.sync.dma_start(wkc[:], Wkc[:].bitcast(F32R))
        for p in range(NP):
            nc.sync.dma_start(xt[p][:], XT[p * 128:(p + 1) * 128, :].bitcast(F32R))
        nc.sync.dma_start(ct[:], CT[:].bitcast(F32R))
        nc.sync.dma_start(wv[:], Wv[:].bitcast(F32R))
        nc.sync.dma_start(wqp[:], Wqp[:].bitcast(F32R))
        nc.sync.dma_start(wqc[:], Wqc[:].bitcast(F32R))
        nc.sync.dma_start(wo[:], Wo[:].bitcast(F32R))

        # ones columns of vtile (in*0 + 1), seeded from xt[0] once it lands
        for c0, c1, w in ((0, 2048, 2048), (2048, 4096, 2048), (4096, NKT * VW, 64)):
            nc.vector.tensor_scalar(vtile[:, c0:c1], xt[0][:, 0:w], 0.0, 1.0,
                                    op0=MULT, op1=ADD)

        def qkproj_pix(t, dest, wmat, pool, tag):
            for g in range(2):
                ps = pool.tile([128, 512], F32, name=tag, tag=tag)
                for p in range(NP):
                    nc.tensor.matmul(
                        ps[:], wmat[:, p * 256 + g * 128:p * 256 + (g + 1) * 128],
                        xt[p][:, t * 512:(t + 1) * 512],
                        start=(p == 0), stop=(p == NP - 1))
                for j in range(2):
                    nc.vector.tensor_scalar(
                        dest[2 * g + j][0:64, t * 512:(t + 1) * 512],
                        ps[j * 64:(j + 1) * 64, :], 5.0, -5.0, op0=MIN, op1=MAX)

        def qkproj_crd(t, dest, wmat, pool, tag):
            for g in range(2):
                ps = pool.tile([128, 512], F32, name=tag, tag=tag)
                nc.tensor.matmul(ps[:], wmat[:, g * 128:(g + 1) * 128],
                                 ct[:, t * 512:(t + 1) * 512], start=True, stop=True)
                for j in range(2):
                    nc.vector.tensor_scalar(
                        dest[2 * g + j][64:128, t * 512:(t + 1) * 512],
                        ps[j * 64:(j + 1) * 64, :], 5.0, -5.0, op0=MIN, op1=MAX)

        # Phase A: K for all t, then V, then Q for t=0 (Q for t>=1 is
        # emitted inside the phase-B loop to start attention earlier).
        with tc.tile_pool(name="pa_qk", bufs=2, space="PSUM") as pqk, \
             tc.tile_pool(name="pa_v", bufs=2, space="PSUM") as pvp:
            for t in range(NT):
                qkproj_pix(t, kcat, wkp, pqk, "pqk")
                qkproj_crd(t, kcat, wkc, pqk, "pqk")
            for t in range(NT):
                for i in range(4):
                    kt = t * 4 + i
                    pv = pvp.tile([128, 256], F32, name="pv", tag="pv")
                    for p in range(NP):
                        nc.tensor.matmul(
                            pv[:], xt[p][:, t * 512 + i * 128:t * 512 + (i + 1) * 128],
                            wv[:, p * 256:(p + 1) * 256],
                            start=(p == 0), stop=(p == NP - 1))
                    for h in range(H_PER_CORE):
                        nc.vector.tensor_copy(
                            vtile[:, kt * VW + h * 65:kt * VW + h * 65 + 64],
                            pv[:, h * 64:(h + 1) * 64])
            qkproj_pix(0, qcat, wqp, pqk, "pqk")
            qkproj_crd(0, qcat, wqc, pqk, "pqk")

        # Phase B: dots -> exp (fused over kt pairs) -> AV with ones column
        # giving the softmax denominator as row 64. Phase C per query chunk.
        with tc.tile_pool(name="pb_s", bufs=2, space="PSUM") as psp, \
             tc.tile_pool(name="pb_o", bufs=2, space="PSUM") as pop, \
             tc.tile_pool(name="pc_y", bufs=2, space="PSUM") as pyp:
            for qi in range(NT):
                for h in range(H_PER_CORE):
                    po = pop.tile([65, 512], F32, name="po", tag="po")
                    for j in range(NKT // 2):
                        k0, k1 = 2 * j, 2 * j + 1
                        ps = psp.tile([128, 1024], F32, name="ps", tag="ps")
                        nc.tensor.matmul(
                            ps[:, 0:512], kcat[h][:, k0 * 128:(k0 + 1) * 128],
                            qcat[h][:, qi * 512:(qi + 1) * 512],
                            start=True, stop=True)
                        nc.tensor.matmul(
                            ps[:, 512:1024], kcat[h][:, k1 * 128:(k1 + 1) * 128],
                            qcat[h][:, qi * 512:(qi + 1) * 512],
                            start=True, stop=True)
                        pe = wk.tile([128, 1024], F32R, name="pe", tag="pe", bufs=3)
                        nc.scalar.activation(pe[:], ps[:], EXP, scale=SCALE)
                        nc.tensor.matmul(
                            po[:], vtile[:, k0 * VW + h * 65:k0 * VW + h * 65 + 65],
                            pe[:, 0:512], start=(j == 0), stop=False)
                        nc.tensor.matmul(
                            po[:], vtile[:, k1 * VW + h * 65:k1 * VW + h * 65 + 65],
                            pe[:, 512:1024], start=False, stop=(j == NKT // 2 - 1))
                    r = wk.tile([1, 512], F32, name="r", tag="r")
                    rb = wk.tile([64, 512], F32, name="rb", tag="rb")
                    nc.vector.reciprocal(r[:], po[64:65, :])
                    nc.gpsimd.partition_broadcast(rb[:], r[:])
                    oj, rr = h // 2, (h % 2) * 64
                    nc.vector.tensor_tensor(
                        ocat[oj][rr:rr + 64, qi * 512:(qi + 1) * 512],
                        po[0:64, :], rb[:], op=MULT)
                for i in range(4):
                    n0 = qi * 512 + i * 128
                    py = pyp.tile([128, 512], F32, name="py", tag="py")
                    nc.tensor.matmul(py[:], ocat[0][:, n0:n0 + 128], wo[:, 0:512],
                                     start=True, stop=False)
                    nc.tensor.matmul(py[:], ocat[1][:, n0:n0 + 128], wo[:, 512:1024],
                                     start=False, stop=True)
                    st = wk.tile([128, 512], F32, name="st", tag="st")
                    nc.vector.tensor_copy(st[:], py[:])
                    nc.sync.dma_start(Y[n0:n0 + 128, :], st[:])
                if qi + 1 < NT:
                    qkproj_pix(qi + 1, qcat, wqp, pyp, "py")
                    qkproj_crd(qi + 1, qcat, wqc, pyp, "py")
    nc.compile()
    return nc


def _get_nc():
    global _NC
    if _NC is None:
        _NC = _build()
    return _NC


def _pack(w, nblk, blk):
    w = np.asarray(w, dtype=np.float32)
    return np.ascontiguousarray(
        w.reshape(nblk, 128, blk).transpose(1, 0, 2).reshape(128, nblk * blk))


def kernel(pixels, coords, mask, W_qkv, W_qkc, W_out, b_out):
    global LAST_EXEC_NS
    pixels = np.asarray(pixels, dtype=np.float32)
    coords = np.asarray(coords, dtype=np.float32)
    W_qkv = np.asarray(W_qkv, dtype=np.float32)
    W_qkc = np.asarray(W_qkc, dtype=np.float32)
    W_out = np.asarray(W_out, dtype=np.float32)
    b_out = np.asarray(b_out, dtype=np.float32)

    nc = _get_nc()

    XT = [np.ascontiguousarray(pixels[b].T) for b in range(B)]
    CT = [np.ascontiguousarray(coords[b].T) for b in range(B)]

    in_maps = []
    for c in range(8):
        b = c // 2
        h0 = (c % 2) * H_PER_CORE * DH     # 0 or 256: col offset within split
        in_maps.append({
            "XT": XT[b],
            "CT": CT[b],
            "Wqp": _pack(W_qkv[:, h0:h0 + 256], 4, 256),
            "Wkp": _pack(W_qkv[:, ID + h0:ID + h0 + 256], 4, 256),
            "Wv": _pack(W_qkv[:, 2 * ID + h0:2 * ID + h0 + 256], 4, 256),
            "Wqc": np.ascontiguousarray(W_qkc[:, h0:h0 + 256]),
            "Wkc": np.ascontiguousarray(W_qkc[:, ID + h0:ID + h0 + 256]),
            "Wo": _pack(W_out[h0:h0 + 256, :], 2, 512),
        })

    res = run_bass_kernel_spmd(nc, in_maps, core_ids=list(range(8)))
    LAST_EXEC_NS = getattr(res, "exec_time_ns", None)

    out = np.empty((B, N, OUT_D), np.float32)
    for b in range(B):
        out[b] = res.results[2 * b]["Y"] + res.results[2 * b + 1]["Y"]
    out += b_out[None, None, :]
    return tuple(np.split(out, [1024], axis=1))


# revision 3
# speedup vs baseline: 1.1627x; 1.1627x over previous
import numpy as np

import concourse.bacc as bacc
import concourse.tile as tile
import concourse.mybir as mybir
from concourse.bass_utils import run_bass_kernel_spmd

F32 = mybir.dt.float32
F32R = mybir.dt.float32r

B = 4
N = 2048
PD = 512
CD = 128
ID = 512
OUT_D = 512
H_PER_CORE = 4
DH = 64
SCALE = 0.125          # dim_head ** -0.5
NT = 4                 # n chunks of 512
NP = 4                 # pd chunks of 128
NKT = 16               # key tiles of 128
VW = 65 * H_PER_CORE   # vtile columns per key tile: 4 x (64 V | 1 one)

_NC = None
LAST_EXEC_NS = None


def _build():
    nc = bacc.Bacc("TRN2", target_bir_lowering=False, debug=False, num_devices=8)
    XT = nc.declare_dram_parameter("XT", [PD, N], F32, isOutput=False)
    CT = nc.declare_dram_parameter("CT", [CD, N], F32, isOutput=False)
    Wqp = nc.declare_dram_parameter("Wqp", [128, 1024], F32, isOutput=False)
    Wkp = nc.declare_dram_parameter("Wkp", [128, 1024], F32, isOutput=False)
    Wv = nc.declare_dram_parameter("Wv", [128, 1024], F32, isOutput=False)
    Wqc = nc.declare_dram_parameter("Wqc", [CD, 256], F32, isOutput=False)
    Wkc = nc.declare_dram_parameter("Wkc", [CD, 256], F32, isOutput=False)
    Wo = nc.declare_dram_parameter("Wo", [128, 1024], F32, isOutput=False)
    Y = nc.declare_dram_parameter("Y", [N, OUT_D], F32, isOutput=True)

    MIN = mybir.AluOpType.min
    MAX = mybir.AluOpType.max
    MULT = mybir.AluOpType.mult
    ADD = mybir.AluOpType.add
    EXP = mybir.ActivationFunctionType.Exp

    with tile.TileContext(nc) as tc, \
         tc.tile_pool(name="persist", bufs=1) as pp, \
         tc.tile_pool(name="work", bufs=2) as wk:
        xt = [pp.tile([128, N], F32R, name=f"xt{p}", tag=f"xt{p}") for p in range(NP)]
        ct = pp.tile([128, N], F32R, name="ct", tag="ct")
        wqp = pp.tile([128, 1024], F32R, name="wqp", tag="wqp")
        wkp = pp.tile([128, 1024], F32R, name="wkp", tag="wkp")
        wv = pp.tile([128, 1024], F32R, name="wv", tag="wv")
        wqc = pp.tile([128, 256], F32R, name="wqc", tag="wqc")
        wkc = pp.tile([128, 256], F32R, name="wkc", tag="wkc")
        wo = pp.tile([128, 1024], F32R, name="wo", tag="wo")
        qcat = [pp.tile([128, N], F32R, name=f"qcat{h}", tag=f"qcat{h}")
                for h in range(H_PER_CORE)]
        kcat = [pp.tile([128, N], F32R, name=f"kcat{h}", tag=f"kcat{h}")
                for h in range(H_PER_CORE)]
        vtile = pp.tile([128, NKT * VW], F32R, name="vtile", tag="vtile")
        ocat = [pp.tile([128, N], F32R, name=f"ocat{j}", tag=f"ocat{j}")
                for j in range(2)]

        nc.sync.dma_start(wkp[:], Wkp[:].bitcast(F32R))
        nc.sync.dma_start(wkc[:], Wkc[:].bitcast(F32R))
        for p in range(NP):
            nc.sync.dma_start(xt[p][:], XT[p * 128:(p + 1) * 128, :].bitcast(F32R))
        nc.sync.dma_start(ct[:], CT[:].bitcast(F32R))
        nc.sync.dma_start(wv[:], Wv[:].bitcast(F32R))
        nc.sync.dma_start(wqp[:], Wqp[:].bitcast(F32R))
        nc.sync.dma_start(wqc[:], Wqc[:].bitcast(F32R))
        nc.sync.dma_start(wo[:], Wo[:].bitcast(F32R))

        # ones columns of vtile (in*0 + 1), seeded from xt[0] once it lands
        for c0, c1, w in ((0, 2048, 2048), (2048, 4096, 2048), (4096, NKT * VW, 64)):
            nc.vector.tensor_scalar(vtile[:, c0:c1], xt[0][:, 0:w], 0.0, 1.0,
                                    op0=MULT, op1=ADD)

        def qkproj_pix(t, dest, wmat, pool, tag):
            for g in range(2):
                ps = pool.tile([128, 512], F32, name=tag, tag=tag)
                for p in range(NP):
                    nc.tensor.matmul(
                        ps[:], wmat[:, p * 256 + g * 128:p * 256 + (g + 1) * 128],
                        xt[p][:, t * 512:(t + 1) * 512],
                        start=(p == 0), stop=(p == NP - 1))
                for j in range(2):
                    nc.vector.tensor_scalar(
                        dest[2 * g + j][0:64, t * 512:(t + 1) * 512],
                        ps[j * 64:(j + 1) * 64, :], 5.0, -5.0, op0=MIN, op1=MAX)

        def qkproj_crd(t, dest, wmat, pool, tag):
            for g in range(2):
                ps = pool.tile([128, 512], F32, name=tag, tag=tag)
                nc.tensor.matmul(ps[:], wmat[:, g * 128:(g + 1) * 128],
                                 ct[:, t * 512:(t + 1) * 512], start=True, stop=True)
                for j in range(2):
                    nc.vector.tensor_scalar(
                        dest[2 * g + j][64:128, t * 512:(t + 1) * 512],
                        ps[j * 64:(j + 1) * 64, :], 5.0, -5.0, op0=MIN, op1=MAX)

        # Phase A: K for all t, then V, then Q for t=0 (Q for t>=1 is
        # emitted inside the phase-B loop to start attention earlier).
        with tc.tile_pool(name="pa_qk", bufs=2, space="PSUM") as pqk, \
             tc.tile_pool(name="pa_v", bufs=2, space="PSUM") as pvp:
            for t in range(NT):
                qkproj_pix(t, kcat, wkp, pqk, "pqk")
                qkproj_crd(t, kcat, wkc, pqk, "pqk")
            for t in range(NT):
                for i in range(4):
                    kt = t * 4 + i
                    pv = pvp.tile([128, 256], F32, name="pv", tag="pv")
                    for p in range(NP):
                        nc.tensor.matmul(
                            pv[:], xt[p][:, t * 512 + i * 128:t * 512 + (i + 1) * 128],
                            wv[:, p * 256:(p + 1) * 256],
                            start=(p == 0), stop=(p == NP - 1))
                    for h in range(H_PER_CORE):
                        nc.vector.tensor_copy(
                            vtile[:, kt * VW + h * 65:kt * VW + h * 65 + 64],
                            pv[:, h * 64:(h + 1) * 64])
            qkproj_pix(0, qcat, wqp, pqk, "pqk")
            qkproj_crd(0, qcat, wqc, pqk, "pqk")

        # Phase B: flat software pipeline over 128 kt-pairs (4 qi x 4 h x 8 j).
        # Steady state per pair g: ACT(exp) of pair g, AV matmuls of pair g,
        # dots matmuls of pair g+2 (so ACT never starves behind the in-order
        # PE queue). Norm chains, phase C (output proj) of the previous qi,
        # and Q projection of the next qi are spread into PE slack.
        NPAIR = NT * H_PER_CORE * (NKT // 2)
        with tc.tile_pool(name="pb_s", bufs=2, space="PSUM") as psp, \
             tc.tile_pool(name="pb_o", bufs=2, space="PSUM") as pop, \
             tc.tile_pool(name="pc_y", bufs=2, space="PSUM") as pyp:
            ps_tiles = {}
            po_tiles = {}
            qstate = {}

            def loc(g):
                qi, r = divmod(g, 32)
                h, j = divmod(r, 8)
                return qi, h, j

            def emit_dots(g):
                qi, h, j = loc(g)
                ps = psp.tile([128, 1024], F32, name="ps", tag="ps")
                ps_tiles[g] = ps
                k0, k1 = 2 * j, 2 * j + 1
                nc.tensor.matmul(
                    ps[:, 0:512], kcat[h][:, k0 * 128:(k0 + 1) * 128],
                    qcat[h][:, qi * 512:(qi + 1) * 512], start=True, stop=True)
                nc.tensor.matmul(
                    ps[:, 512:1024], kcat[h][:, k1 * 128:(k1 + 1) * 128],
                    qcat[h][:, qi * 512:(qi + 1) * 512], start=True, stop=True)

            def emit_av(g, pe):
                qi, h, j = loc(g)
                if j == 0:
                    po_tiles[(qi, h)] = pop.tile([65, 512], F32, name="po", tag="po")
                po = po_tiles[(qi, h)]
                k0, k1 = 2 * j, 2 * j + 1
                nc.tensor.matmul(
                    po[:], vtile[:, k0 * VW + h * 65:k0 * VW + h * 65 + 65],
                    pe[:, 0:512], start=(j == 0), stop=False)
                nc.tensor.matmul(
                    po[:], vtile[:, k1 * VW + h * 65:k1 * VW + h * 65 + 65],
                    pe[:, 512:1024], start=False, stop=(j == NKT // 2 - 1))

            def emit_norm(qi, h):
                po = po_tiles[(qi, h)]
                r = wk.tile([1, 512], F32, name="r", tag="r")
                rb = wk.tile([64, 512], F32, name="rb", tag="rb")
                nc.vector.reciprocal(r[:], po[64:65, :])
                nc.gpsimd.partition_broadcast(rb[:], r[:])
                oj, rr = h // 2, (h % 2) * 64
                nc.vector.tensor_tensor(
                    ocat[oj][rr:rr + 64, qi * 512:(qi + 1) * 512],
                    po[0:64, :], rb[:], op=MULT)

            def emit_phasec(qi, i):
                n0 = qi * 512 + i * 128
                py = pyp.tile([128, 512], F32, name="py", tag="py")
                nc.tensor.matmul(py[:], ocat[0][:, n0:n0 + 128], wo[:, 0:512],
                                 start=True, stop=False)
                nc.tensor.matmul(py[:], ocat[1][:, n0:n0 + 128], wo[:, 512:1024],
                                 start=False, stop=True)
                st = wk.tile([128, 512], F32, name="st", tag="st")
                nc.vector.tensor_copy(st[:], py[:])
                nc.sync.dma_start(Y[n0:n0 + 128, :], st[:])

            def emit_qproj_piece(t, p):
                # p 0..7: pixel-part matmul (group p//4, x chunk p%4);
                # p 8..9: coord-part matmul for group p-8. Clamps follow the
                # final accumulating matmul (they run on DVE, not PE).
                if p < 8:
                    g, c = divmod(p, 4)
                    if c == 0:
                        qstate[(t, g)] = pyp.tile([128, 512], F32,
                                                  name="py", tag="py")
                    ps = qstate[(t, g)]
                    nc.tensor.matmul(
                        ps[:], wqp[:, c * 256 + g * 128:c * 256 + (g + 1) * 128],
                        xt[c][:, t * 512:(t + 1) * 512],
                        start=(c == 0), stop=(c == 3))
                    if c == 3:
                        for jj in range(2):
                            nc.vector.tensor_scalar(
                                qcat[2 * g + jj][0:64, t * 512:(t + 1) * 512],
                                ps[jj * 64:(jj + 1) * 64, :], 5.0, -5.0,
                                op0=MIN, op1=MAX)
                else:
                    g = p - 8
                    ps = pyp.tile([128, 512], F32, name="py", tag="py")
                    nc.tensor.matmul(ps[:], wqc[:, g * 128:(g + 1) * 128],
                                     ct[:, t * 512:(t + 1) * 512],
                                     start=True, stop=True)
                    for jj in range(2):
                        nc.vector.tensor_scalar(
                            qcat[2 * g + jj][64:128, t * 512:(t + 1) * 512],
                            ps[jj * 64:(jj + 1) * 64, :], 5.0, -5.0,
                            op0=MIN, op1=MAX)

            emit_dots(0)
            emit_dots(1)
            for g in range(NPAIR):
                qi, h, j = loc(g)
                pe = wk.tile([128, 1024], F32R, name="pe", tag="pe", bufs=3)
                nc.scalar.activation(pe[:], ps_tiles[g][:], EXP, scale=SCALE)
                emit_av(g, pe)
                if g + 2 < NPAIR:
                    emit_dots(g + 2)
                if j == NKT // 2 - 1:
                    emit_norm(qi, h)
                r32 = g % 32
                if qi >= 1 and r32 in (2, 6, 10, 14):
                    emit_phasec(qi - 1, (r32 - 2) // 4)
                if qi + 1 < NT and 16 <= r32 <= 25:
                    emit_qproj_piece(qi + 1, r32 - 16)
            for i in range(4):
                emit_phasec(NT - 1, i)
    nc.compile()
    return nc


def _get_nc():
    global _NC
    if _NC is None:
        _NC = _build()
    return _NC


def _pack(w, nblk, blk):
    w = np.asarray(w, dtype=np.float32)
    return np.ascontiguousarray(
        w.reshape(nblk, 128, blk).transpose(1, 0, 2).reshape(128, nblk * blk))


def kernel(pixels, coords, mask, W_qkv, W_qkc, W_out, b_out):
    global LAST_EXEC_NS
    pixels = np.asarray(pixels, dtype=np.float32)
    coords = np.asarray(coords, dtype=np.float32)
    W_qkv = np.asarray(W_qkv, dtype=np.float32)
    W_qkc = np.asarray(W_qkc, dtype=np.float32)
    W_out = np.asarray(W_out, dtype=np.float32)
    b_out = np.asarray(b_out, dtype=np.float32)

    nc = _get_nc()

    XT = [np.ascontiguousarray(pixels[b].T) for b in range(B)]
    CT = [np.ascontiguousarray(coords[b].T) for b in range(B)]

    in_maps = []
    for c in range(8):
        b = c // 2
        h0 = (c % 2) * H_PER_CORE * DH     # 0 or 256: col offset within split
        in_maps.append({
            "XT": XT[b],
            "CT": CT[b],
            "Wqp": _pack(W_qkv[:, h0:h0 + 256], 4, 256),
            "Wkp": _pack(W_qkv[:, ID + h0:ID + h0 + 256], 4, 256),
            "Wv": _pack(W_qkv[:, 2 * ID + h0:2 * ID + h0 + 256], 4, 256),
            "Wqc": np.ascontiguousarray(W_qkc[:, h0:h0 + 256]),
            "Wkc": np.ascontiguousarray(W_qkc[:, ID + h0:ID + h0 + 256]),
            "Wo": _pack(W_out[h0:h0 + 256, :], 2, 512),
        })

    res = run_bass_kernel_spmd(nc, in_maps, core_ids=list(range(8)))
    LAST_EXEC_NS = getattr(res, "exec_time_ns", None)

    out = np.empty((B, N, OUT_D), np.float32)
    for b in range(B):
        out[b] = res.results[2 * b]["Y"] + res.results[2 * b + 1]["Y"]
    out += b_out[None, None, :]
    return tuple(np.split(out, [1024], axis=1))


# revision 5
# speedup vs baseline: 1.2068x; 1.0380x over previous
import numpy as np

import concourse.bacc as bacc
import concourse.tile as tile
import concourse.mybir as mybir
from concourse.bass_utils import run_bass_kernel_spmd

F32 = mybir.dt.float32
F32R = mybir.dt.float32r

B = 4
N = 2048
PD = 512
CD = 128
ID = 512
OUT_D = 512
H_PER_CORE = 4
DH = 64
SCALE = 0.125          # dim_head ** -0.5
NT = 4                 # n chunks of 512
NP = 4                 # pd chunks of 128
NKT = 16               # key tiles of 128
VW = 65 * H_PER_CORE   # vtile columns per key tile: 4 x (64 V | 1 one)

_NC = None
LAST_EXEC_NS = None


def _build():
    nc = bacc.Bacc("TRN2", target_bir_lowering=False, debug=False, num_devices=8)
    XT = nc.declare_dram_parameter("XT", [PD, N], F32, isOutput=False)
    CT = nc.declare_dram_parameter("CT", [CD, N], F32, isOutput=False)
    Wqp = nc.declare_dram_parameter("Wqp", [128, 1024], F32, isOutput=False)
    Wkp = nc.declare_dram_parameter("Wkp", [128, 1024], F32, isOutput=False)
    Wv = nc.declare_dram_parameter("Wv", [128, 1024], F32, isOutput=False)
    Wqc = nc.declare_dram_parameter("Wqc", [CD, 256], F32, isOutput=False)
    Wkc = nc.declare_dram_parameter("Wkc", [CD, 256], F32, isOutput=False)
    Wo = nc.declare_dram_parameter("Wo", [128, 1024], F32, isOutput=False)
    Y = nc.declare_dram_parameter("Y", [N, OUT_D], F32, isOutput=True)

    MIN = mybir.AluOpType.min
    MAX = mybir.AluOpType.max
    MULT = mybir.AluOpType.mult
    ADD = mybir.AluOpType.add
    EXP = mybir.ActivationFunctionType.Exp

    with tile.TileContext(nc) as tc, \
         tc.tile_pool(name="persist", bufs=1) as pp, \
         tc.tile_pool(name="work", bufs=2) as wk, \
         tc.tile_pool(name="pb_s", bufs=2, space="PSUM") as psp, \
         tc.tile_pool(name="pb_o", bufs=2, space="PSUM") as pop, \
         tc.tile_pool(name="pc_y", bufs=2, space="PSUM") as pyp:
        xt = [pp.tile([128, N], F32R, name=f"xt{p}", tag=f"xt{p}") for p in range(NP)]
        ct = pp.tile([128, N], F32R, name="ct", tag="ct")
        wqp = pp.tile([128, 1024], F32R, name="wqp", tag="wqp")
        wkp = pp.tile([128, 1024], F32R, name="wkp", tag="wkp")
        wv = pp.tile([128, 1024], F32R, name="wv", tag="wv")
        wqc = pp.tile([128, 256], F32R, name="wqc", tag="wqc")
        wkc = pp.tile([128, 256], F32R, name="wkc", tag="wkc")
        wo = pp.tile([128, 1024], F32R, name="wo", tag="wo")
        qcat = [pp.tile([128, N], F32R, name=f"qcat{h}", tag=f"qcat{h}")
                for h in range(H_PER_CORE)]
        kcat = [pp.tile([128, N], F32R, name=f"kcat{h}", tag=f"kcat{h}")
                for h in range(H_PER_CORE)]
        vtile = pp.tile([128, NKT * VW], F32R, name="vtile", tag="vtile")
        ocat = [pp.tile([128, N], F32R, name=f"ocat{j}", tag=f"ocat{j}")
                for j in range(2)]

        # Input DMAs split across two queues (SP + Pool doorbell) so the
        # critical prologue tensors land sooner.
        nc.sync.dma_start(wkp[:], Wkp[:].bitcast(F32R))
        nc.gpsimd.dma_start(wkc[:], Wkc[:].bitcast(F32R))
        nc.sync.dma_start(wqp[:], Wqp[:].bitcast(F32R))
        nc.gpsimd.dma_start(wqc[:], Wqc[:].bitcast(F32R))
        nc.sync.dma_start(xt[0][:], XT[0:128, :].bitcast(F32R))
        nc.gpsimd.dma_start(xt[2][:], XT[256:384, :].bitcast(F32R))
        nc.sync.dma_start(xt[1][:], XT[128:256, :].bitcast(F32R))
        nc.gpsimd.dma_start(xt[3][:], XT[384:512, :].bitcast(F32R))
        nc.sync.dma_start(ct[:], CT[:].bitcast(F32R))
        nc.gpsimd.dma_start(wv[:], Wv[:].bitcast(F32R))
        nc.sync.dma_start(wo[:], Wo[:].bitcast(F32R))

        # Everything (QKV projections, attention, output projection) runs in
        # one flat software pipeline over 128 kt-pairs (4 qi x 4 h x 8 j).
        # Steady state per pair g: ACT(exp) of pair g, AV matmuls of pair g,
        # dots matmuls of pair g+2 (so ACT never starves behind the in-order
        # PE queue). K/V/Q projections, norm chains, and the output
        # projection are spread into PE slack with emission-order deadlines.
        NPAIR = NT * H_PER_CORE * (NKT // 2)
        if True:
            ps_tiles = {}
            po_tiles = {}
            qstate = {}

            def proj_tg(t, g, wpix, wcrd, dest):
                # qk projection for column block t, head-group g (heads
                # 2g, 2g+1): pixel part into rows 0:64, coord into 64:128.
                ps = pyp.tile([128, 512], F32, name="py", tag="py")
                for c in range(NP):
                    nc.tensor.matmul(
                        ps[:], wpix[:, c * 256 + g * 128:c * 256 + (g + 1) * 128],
                        xt[c][:, t * 512:(t + 1) * 512],
                        start=(c == 0), stop=(c == NP - 1))
                for jj in range(2):
                    nc.vector.tensor_scalar(
                        dest[2 * g + jj][0:64, t * 512:(t + 1) * 512],
                        ps[jj * 64:(jj + 1) * 64, :], 5.0, -5.0, op0=MIN, op1=MAX)
                ps2 = pyp.tile([128, 512], F32, name="py", tag="py")
                nc.tensor.matmul(ps2[:, 0:512], wcrd[:, g * 128:(g + 1) * 128],
                                 ct[:, t * 512:(t + 1) * 512], start=True, stop=True)
                for jj in range(2):
                    nc.vector.tensor_scalar(
                        dest[2 * g + jj][64:128, t * 512:(t + 1) * 512],
                        ps2[jj * 64:(jj + 1) * 64, :], 5.0, -5.0, op0=MIN, op1=MAX)

            def v_kt(kt):
                t, i = divmod(kt, 4)
                pv = pyp.tile([128, 512], F32, name="py", tag="py")
                for p in range(NP):
                    nc.tensor.matmul(
                        pv[:, 0:256],
                        xt[p][:, t * 512 + i * 128:t * 512 + (i + 1) * 128],
                        wv[:, p * 256:(p + 1) * 256],
                        start=(p == 0), stop=(p == NP - 1))
                for h in range(H_PER_CORE):
                    nc.vector.tensor_copy(
                        vtile[:, kt * VW + h * 65:kt * VW + h * 65 + 64],
                        pv[:, h * 64:(h + 1) * 64])

            def loc(g):
                qi, r = divmod(g, 32)
                h, j = divmod(r, 8)
                return qi, h, j

            def emit_dots(g):
                qi, h, j = loc(g)
                ps = psp.tile([128, 1024], F32, name="ps", tag="ps")
                ps_tiles[g] = ps
                k0, k1 = 2 * j, 2 * j + 1
                nc.tensor.matmul(
                    ps[:, 0:512], kcat[h][:, k0 * 128:(k0 + 1) * 128],
                    qcat[h][:, qi * 512:(qi + 1) * 512], start=True, stop=True)
                nc.tensor.matmul(
                    ps[:, 512:1024], kcat[h][:, k1 * 128:(k1 + 1) * 128],
                    qcat[h][:, qi * 512:(qi + 1) * 512], start=True, stop=True)

            def emit_av(g, pe):
                qi, h, j = loc(g)
                if j == 0:
                    po_tiles[(qi, h)] = pop.tile([65, 512], F32, name="po", tag="po")
                po = po_tiles[(qi, h)]
                k0, k1 = 2 * j, 2 * j + 1
                nc.tensor.matmul(
                    po[:], vtile[:, k0 * VW + h * 65:k0 * VW + h * 65 + 65],
                    pe[:, 0:512], start=(j == 0), stop=False)
                nc.tensor.matmul(
                    po[:], vtile[:, k1 * VW + h * 65:k1 * VW + h * 65 + 65],
                    pe[:, 512:1024], start=False, stop=(j == NKT // 2 - 1))

            def emit_norm(qi, h):
                po = po_tiles[(qi, h)]
                r = wk.tile([1, 512], F32, name="r", tag="r")
                rb = wk.tile([64, 512], F32, name="rb", tag="rb")
                nc.vector.reciprocal(r[:], po[64:65, :])
                nc.gpsimd.partition_broadcast(rb[:], r[:])
                oj, rr = h // 2, (h % 2) * 64
                nc.vector.tensor_tensor(
                    ocat[oj][rr:rr + 64, qi * 512:(qi + 1) * 512],
                    po[0:64, :], rb[:], op=MULT)

            def emit_phasec(qi, i):
                n0 = qi * 512 + i * 128
                py = pyp.tile([128, 512], F32, name="py", tag="py")
                nc.tensor.matmul(py[:], ocat[0][:, n0:n0 + 128], wo[:, 0:512],
                                 start=True, stop=False)
                nc.tensor.matmul(py[:], ocat[1][:, n0:n0 + 128], wo[:, 512:1024],
                                 start=False, stop=True)
                st = wk.tile([128, 512], F32, name="st", tag="st")
                nc.vector.tensor_copy(st[:], py[:])
                nc.sync.dma_start(Y[n0:n0 + 128, :], st[:])

            def emit_qproj_piece(t, p):
                # p 0..7: pixel-part matmul (group p//4, x chunk p%4);
                # p 8..9: coord-part matmul for group p-8. Clamps follow the
                # final accumulating matmul (they run on DVE, not PE).
                if p < 8:
                    g, c = divmod(p, 4)
                    if c == 0:
                        qstate[(t, g)] = pyp.tile([128, 512], F32,
                                                  name="py", tag="py")
                    ps = qstate[(t, g)]
                    nc.tensor.matmul(
                        ps[:], wqp[:, c * 256 + g * 128:c * 256 + (g + 1) * 128],
                        xt[c][:, t * 512:(t + 1) * 512],
                        start=(c == 0), stop=(c == 3))
                    if c == 3:
                        for jj in range(2):
                            nc.vector.tensor_scalar(
                                qcat[2 * g + jj][0:64, t * 512:(t + 1) * 512],
                                ps[jj * 64:(jj + 1) * 64, :], 5.0, -5.0,
                                op0=MIN, op1=MAX)
                else:
                    g = p - 8
                    ps = pyp.tile([128, 512], F32, name="py", tag="py")
                    nc.tensor.matmul(ps[:], wqc[:, g * 128:(g + 1) * 128],
                                     ct[:, t * 512:(t + 1) * 512],
                                     start=True, stop=True)
                    for jj in range(2):
                        nc.vector.tensor_scalar(
                            qcat[2 * g + jj][64:128, t * 512:(t + 1) * 512],
                            ps[jj * 64:(jj + 1) * 64, :], 5.0, -5.0,
                            op0=MIN, op1=MAX)

            def ones_cols(c0, c1, w):
                # vtile ones-columns: in*0 + 1, seeded from xt[0]
                nc.vector.tensor_scalar(vtile[:, c0:c1], xt[0][:, 0:w], 0.0, 1.0,
                                        op0=MULT, op1=ADD)

            # Prologue: minimal K/Q prefix so exp can start ASAP, then the
            # V tiles and K columns needed by the first few pipeline pairs.
            proj_tg(0, 0, wkp, wkc, kcat)
            proj_tg(0, 0, wqp, wqc, qcat)
            emit_dots(0)
            emit_dots(1)
            ones_cols(0, 2048, 2048)
            for kt in range(4):
                v_kt(kt)
            proj_tg(1, 0, wkp, wkc, kcat)
            ones_cols(2048, 4096, 2048)
            ones_cols(4096, NKT * VW, 64)
            for kt in range(4, 8):
                v_kt(kt)

            # Filler pieces for the qi=0 block, keyed by pipeline position;
            # each fits the ~1us PE slack without starving ACT, and lands
            # before its first consumer's emission point.
            fill0 = {
                0: [lambda: proj_tg(2, 0, wkp, wkc, kcat), lambda: v_kt(8)],
                1: [lambda: v_kt(9), lambda: v_kt(10)],
                2: [lambda: proj_tg(3, 0, wkp, wkc, kcat), lambda: v_kt(11)],
                3: [lambda: v_kt(12), lambda: v_kt(13)],
                4: [lambda: v_kt(14), lambda: v_kt(15)],
                5: [lambda: proj_tg(0, 1, wkp, wkc, kcat)],
                6: [lambda: proj_tg(1, 1, wkp, wkc, kcat)],
                7: [lambda: proj_tg(2, 1, wkp, wkc, kcat)],
                8: [lambda: proj_tg(3, 1, wkp, wkc, kcat)],
                9: [lambda: proj_tg(0, 1, wqp, wqc, qcat)],
            }

            for g in range(NPAIR):
                qi, h, j = loc(g)
                pe = wk.tile([128, 1024], F32R, name="pe", tag="pe", bufs=3)
                nc.scalar.activation(pe[:], ps_tiles[g][:], EXP, scale=SCALE)
                emit_av(g, pe)
                if g + 2 < NPAIR:
                    emit_dots(g + 2)
                r32 = g % 32
                if qi == 0 and r32 in fill0:
                    for f in fill0[r32]:
                        f()
                if qi >= 1 and r32 in (2, 6, 10, 14):
                    emit_phasec(qi - 1, (r32 - 2) // 4)
                if qi + 1 < NT and 16 <= r32 <= 25:
                    emit_qproj_piece(qi + 1, r32 - 16)
                if j == NKT // 2 - 1:
                    emit_norm(qi, h)
            for i in range(4):
                emit_phasec(NT - 1, i)
    nc.compile()
    return nc


def _get_nc():
    global _NC
    if _NC is None:
        _NC = _build()
    return _NC


def _pack(w, nblk, blk):
    w = np.asarray(w, dtype=np.float32)
    return np.ascontiguousarray(
        w.reshape(nblk, 128, blk).transpose(1, 0, 2).reshape(128, nblk * blk))


def kernel(pixels, coords, mask, W_qkv, W_qkc, W_out, b_out):
    global LAST_EXEC_NS
    pixels = np.asarray(pixels, dtype=np.float32)
    coords = np.asarray(coords, dtype=np.float32)
    W_qkv = np.asarray(W_qkv, dtype=np.float32)
    W_qkc = np.asarray(W_qkc, dtype=np.float32)
    W_out = np.asarray(W_out, dtype=np.float32)
    b_out = np.asarray(b_out, dtype=np.float32)

    nc = _get_nc()

    XT = [np.ascontiguousarray(pixels[b].T) for b in range(B)]
    CT = [np.ascontiguousarray(coords[b].T) for b in range(B)]

    in_maps = []
    for c in range(8):
        b = c // 2
        h0 = (c % 2) * H_PER_CORE * DH     # 0 or 256: col offset within split
        in_maps.append({
            "XT": XT[b],
            "CT": CT[b],
            "Wqp": _pack(W_qkv[:, h0:h0 + 256], 4, 256),
            "Wkp": _pack(W_qkv[:, ID + h0:ID + h0 + 256], 4, 256),
            "Wv": _pack(W_qkv[:, 2 * ID + h0:2 * ID + h0 + 256], 4, 256),
            "Wqc": np.ascontiguousarray(W_qkc[:, h0:h0 + 256]),
            "Wkc": np.ascontiguousarray(W_qkc[:, ID + h0:ID + h0 + 256]),
            "Wo": _pack(W_out[h0:h0 + 256, :], 2, 512),
        })

    res = run_bass_kernel_spmd(nc, in_maps, core_ids=list(range(8)))
    LAST_EXEC_NS = getattr(res, "exec_time_ns", None)

    out = np.empty((B, N, OUT_D), np.float32)
    for b in range(B):
        out[b] = res.results[2 * b]["Y"] + res.results[2 * b + 1]["Y"]
    out += b_out[None, None, :]
    return tuple(np.split(out, [1024], axis=1))
